# revision 9
# baseline (speedup 1.0000x reference)
"""Trainium2 Bass kernel for an 8-head self-attention block (MHA).

Problem: x[2, 4096, 512], 8 heads x 64 dims, torch-Linear q/k/v/o projections,
softmax attention, residual:  out = softmax(q k^T / 8) v @ Wo^T + bo + x.

Sharding (8 NeuronCores, no collectives): core c handles batch b = c // 4 and
query rows (c % 4) * 1024 ... + 1024, for ALL heads.  K/V for the full
sequence are computed on every core of a batch group (projections are cheap
relative to attention), so the output projection is fully local to a core.

Layouts are chosen so no on-device transpose is ever needed; the host passes
x^T and pre-transposed weights:
  - kT[f, s] f32 in SBUF; scores lhsT slices [64, 128]
  - qT[f, q] f32, pre-scaled by 1/sqrt(64); scores rhs slices [64, 512]
  - scores^T chunk [s=128, q=512] = kT_sl.T @ qT_sl on PE (psum)
  - exp on ACT -> P~ bf16 (no max subtraction: scores are O(1) here)
  - V bf16, s-chunk-major with per-head-pair 130-col blocks containing
    constant-1 columns, so each PV matmul also accumulates the softmax
    denominator into an adjacent psum row (even head: rows 0-63 data /
    row 64 denom; odd head: row 63 denom / rows 64-127 data)
  - normalize: DVE reciprocal of denom row, K=1 ones-outer matmul to
    broadcast it across partitions, DVE multiply -> oT[f, q] f32
  - output projection contracts oT with Wo^T; bias enters as a K=1
    outer-product matmul into the same psum; residual added on DVE.
"""

import numpy as np

B = 2
S = 4096
E = 512
H = 8
D = 64
P = 128
EC = E // P          # 4 e-chunks
FC = E // P          # 4 f-chunks
NJ = S // P          # 32 s-chunks
QR = S // 4          # 1024 query rows per core
NQS = QR // 512      # 2 query strips of 512
NKS = S // 512       # 8 s-strips of 512

_CACHE = {}


def _build_nc():
    import concourse.bass as bass
    import concourse.tile as tile
    from concourse import bacc, mybir

    f32 = mybir.dt.float32
    bf16 = mybir.dt.bfloat16
    AFT = mybir.ActivationFunctionType
    Alu = mybir.AluOpType

    nc = bacc.Bacc("TRN2", target_bir_lowering=False, debug=False, num_devices=8)

    xT_d = nc.declare_dram_parameter("xT", [E, S], f32, isOutput=False)
    xqT_d = nc.declare_dram_parameter("xqT", [E, QR], f32, isOutput=False)
    xres_d = nc.declare_dram_parameter("xres", [QR, E], f32, isOutput=False)
    wqT_d = nc.declare_dram_parameter("wqT", [E, E], f32, isOutput=False)
    wkT_d = nc.declare_dram_parameter("wkT", [E, E], f32, isOutput=False)
    wvT_d = nc.declare_dram_parameter("wvT", [E, E], f32, isOutput=False)
    woT_d = nc.declare_dram_parameter("woT", [E, E], f32, isOutput=False)
    bq_d = nc.declare_dram_parameter("bq", [P, FC], f32, isOutput=False)
    bk_d = nc.declare_dram_parameter("bk", [P, FC], f32, isOutput=False)
    bv_d = nc.declare_dram_parameter("bv", [E], f32, isOutput=False)
    bo_d = nc.declare_dram_parameter("bo", [E], f32, isOutput=False)
    ones_d = nc.declare_dram_parameter("ones", [1, P], f32, isOutput=False)
    out_d = nc.declare_dram_parameter("out", [QR, E], f32, isOutput=True)

    with tile.TileContext(nc) as tc:
        with tc.tile_pool(name="const", bufs=1) as const, \
             tc.tile_pool(name="persist", bufs=1) as persist:

            # ---- constants that live for the whole kernel ----
            wo_sb = const.tile([P, EC, E], f32)
            nc.sync.dma_start(
                out=wo_sb[:], in_=woT_d.ap().rearrange("(c p) f -> p c f", p=P))
            bq_sb = const.tile([P, FC], f32)
            bk_sb = const.tile([P, FC], f32)
            nc.sync.dma_start(out=bq_sb[:], in_=bq_d[:])
            nc.sync.dma_start(out=bk_sb[:], in_=bk_d[:])
            bv_sb = const.tile([P, E], f32)
            nc.sync.dma_start(
                out=bv_sb[:],
                in_=bass.AP(tensor=bv_d, offset=0, ap=[[0, P], [1, E]]))
            bo_sb = const.tile([1, E], f32)
            nc.sync.dma_start(
                out=bo_sb[:],
                in_=bass.AP(tensor=bo_d, offset=0, ap=[[0, 1], [1, E]]))
            # ones replicated on every partition so K=1 outer-product
            # matmuls can pick a lhsT row at any base partition
            ones_sb = const.tile([P, P], f32)
            nc.sync.dma_start(
                out=ones_sb[:],
                in_=bass.AP(tensor=ones_d, offset=0, ap=[[0, P], [1, P]]))

            # ---- persistent activations ----
            kT_sb = persist.tile([P, FC, S], f32)            # 64 KB/p
            qT_sb = persist.tile([P, FC, QR], f32)           # 16 KB/p
            v_sb = persist.tile([P, NJ, H, 65], bf16)        # 32.5 KB/p
            oT_sb = persist.tile([P, FC, QR], f32)           # 16 KB/p

            # constant-1 columns (softmax denominator trick)
            nc.vector.memset(v_sb[:, :, :, 64:65], 1.0)

            # ================= phase B: projections =================
            with tc.tile_pool(name="wpool", bufs=1) as wpool, \
                 tc.tile_pool(name="xtp", bufs=2) as xtp, \
                 tc.tile_pool(name="ps_b", bufs=3, space="PSUM") as ps_b:

                wq_sb = wpool.tile([P, EC, E], f32)
                wk_sb = wpool.tile([P, EC, E], f32)
                wv_sb = wpool.tile([P, EC, E], f32)
                for t, d in ((wq_sb, wqT_d), (wk_sb, wkT_d), (wv_sb, wvT_d)):
                    nc.sync.dma_start(
                        out=t[:], in_=d.ap().rearrange("(c p) f -> p c f", p=P))

                # B1: kT[f, s] = Wk @ x^T (+ bk), streaming xT strips
                for strip in range(NKS):
                    ssl = slice(strip * 512, (strip + 1) * 512)
                    xt = xtp.tile([P, EC, 512], f32, tag="xt")
                    for e in range(EC):
                        nc.sync.dma_start(
                            out=xt[:, e, :], in_=xT_d[e * P:(e + 1) * P, ssl])
                    for f in range(FC):
                        pk = ps_b.tile([P, 512], f32, tag="pb")
                        for e in range(EC):
                            nc.tensor.matmul(
                                pk[:], wk_sb[:, e, f * P:(f + 1) * P],
                                xt[:, e, :], start=(e == 0), stop=(e == EC - 1),
                                skip_group_check=True)
                        nc.vector.tensor_scalar_add(
                            kT_sb[:, f, ssl], pk[:], bk_sb[:, f:f + 1])

                # B2: qT[f, q] = (Wq @ xq^T + bq) / 8, from the query slice
                for qs in range(NQS):
                    qsl = slice(qs * 512, (qs + 1) * 512)
                    xq = xtp.tile([P, EC, 512], f32, tag="xt")
                    for e in range(EC):
                        nc.sync.dma_start(
                            out=xq[:, e, :], in_=xqT_d[e * P:(e + 1) * P, qsl])
                    for f in range(FC):
                        pq = ps_b.tile([P, 512], f32, tag="pb")
                        for e in range(EC):
                            nc.tensor.matmul(
                                pq[:], wq_sb[:, e, f * P:(f + 1) * P],
                                xq[:, e, :], start=(e == 0), stop=(e == EC - 1),
                                skip_group_check=True)
                        nc.vector.tensor_scalar(
                            qT_sb[:, f, qsl], pq[:], bq_sb[:, f:f + 1],
                            float(1.0 / np.sqrt(D)), Alu.add, Alu.mult)

                # B3: V (natural layout, bf16, interleaved with 1-columns)
                for j in range(NJ):
                    jsl = slice(j * P, (j + 1) * P)
                    xv = xtp.tile([P, EC, P], f32, tag="xv")
                    for e in range(EC):
                        nc.sync.dma_start(
                            out=xv[:, e, :], in_=xT_d[e * P:(e + 1) * P, jsl])
                    pv = ps_b.tile([P, E], f32, tag="pb")
                    for e in range(EC):
                        nc.tensor.matmul(
                            pv[:], xv[:, e, :], wv_sb[:, e, :],
                            start=(e == 0), stop=(e == EC - 1),
                            skip_group_check=True)
                    pv_v = pv[:].rearrange("p (h d) -> p h d", h=H)
                    bv_v = bv_sb[:].rearrange("p (h d) -> p h d", h=H)
                    nc.vector.tensor_add(v_sb[:, j, :, 0:64], pv_v[:], bv_v[:])

            # ================= phase C: attention =================
            with tc.tile_pool(name="work", bufs=4) as work, \
                 tc.tile_pool(name="rec", bufs=2) as rec, \
                 tc.tile_pool(name="ps_sc", bufs=3, space="PSUM") as ps_sc, \
                 tc.tile_pool(name="ps_pv", bufs=2, space="PSUM") as ps_pv, \
                 tc.tile_pool(name="ps_bc", bufs=1, space="PSUM") as ps_bc:

                for h in range(H):
                    fc = h // 2
                    fr = (h % 2) * 64
                    even = (h % 2) == 0
                    for qs in range(NQS):
                        qsl = slice(qs * 512, (qs + 1) * 512)
                        pvp = ps_pv.tile([P, 512], f32, tag="pv")
                        for j in range(NJ):
                            sc = ps_sc.tile([P, 512], f32, tag="sc")
                            nc.tensor.matmul(
                                sc[:],
                                kT_sb[fr:fr + 64, fc, j * P:(j + 1) * P],
                                qT_sb[fr:fr + 64, fc, qsl],
                                start=True, stop=True, skip_group_check=True)
                            pt = work.tile([P, 512], bf16, tag="pt")
                            nc.scalar.activation(pt[:], sc[:], AFT.Exp)
                            nc.tensor.matmul(
                                pvp[0:65, :], v_sb[:, j, h, :], pt[:],
                                start=(j == 0), stop=(j == NJ - 1),
                                skip_group_check=True)
                        # normalize by the accumulated denominator row
                        rc = rec.tile([P, 512], f32, tag="rc")
                        nc.vector.reciprocal(rc[64:65, :], pvp[64:65, :])
                        bc = ps_bc.tile([P, 512], f32, tag="bc")
                        nc.tensor.matmul(bc[0:64, :], ones_sb[64:65, 0:64],
                                         rc[64:65, :], start=True, stop=True,
                                         skip_group_check=True)
                        # DVE reads at most one PSUM operand: stage PV data
                        # in SBUF first
                        st = rec.tile([64, 512], f32, tag="st")
                        nc.vector.tensor_copy(st[:], pvp[0:64, :])
                        if even:
                            nc.vector.tensor_mul(oT_sb[0:64, fc, qsl],
                                                 st[:], bc[0:64, :])
                        else:
                            # engines cannot shift partitions; multiply at
                            # base 0 and DMA (which can) into rows 64-127
                            st2 = rec.tile([64, 512], f32, tag="st2")
                            nc.vector.tensor_mul(st2[:], st[:], bc[0:64, :])
                            nc.sync.dma_start(out=oT_sb[64:128, fc, qsl],
                                              in_=st2[:])

            # ============ phase D: output projection + residual ============
            with tc.tile_pool(name="outp", bufs=2) as outp, \
                 tc.tile_pool(name="ps_o", bufs=2, space="PSUM") as ps_o:
                for qc in range(QR // P):
                    po = ps_o.tile([P, E], f32, tag="po")
                    nc.tensor.matmul(po[:], ones_sb[0:1, :], bo_sb[:],
                                     start=True, stop=False,
                                     skip_group_check=True)
                    for e in range(EC):
                        nc.tensor.matmul(
                            po[:], oT_sb[:, e, qc * P:(qc + 1) * P],
                            wo_sb[:, e, :], start=False, stop=(e == EC - 1),
                            skip_group_check=True)
                    xr = outp.tile([P, E], f32, tag="xr")
                    nc.sync.dma_start(
                        out=xr[:], in_=xres_d[qc * P:(qc + 1) * P, :])
                    ot = outp.tile([P, E], f32, tag="ot")
                    nc.vector.tensor_add(ot[:], po[:], xr[:])
                    nc.sync.dma_start(out=out_d[qc * P:(qc + 1) * P, :],
                                      in_=ot[:])

    nc.compile()
    return nc


def _get_nc():
    if "nc" not in _CACHE:
        _CACHE["nc"] = _build_nc()
    return _CACHE["nc"]


def run_spmd(in_maps, **kw):
    from concourse.bass_utils import run_bass_kernel_spmd
    nc = _get_nc()
    return run_bass_kernel_spmd(nc, in_maps, list(range(8)), **kw)


def make_in_maps(x, Wq, bq, Wk, bk, Wv, bv, Wo, bo):
    x = np.asarray(x, dtype=np.float32)
    f32c = lambda a: np.ascontiguousarray(np.asarray(a, dtype=np.float32))
    wqT = f32c(np.asarray(Wq).T)
    wkT = f32c(np.asarray(Wk).T)
    wvT = f32c(np.asarray(Wv).T)
    woT = f32c(np.asarray(Wo).T)
    bq_r = f32c(np.asarray(bq).reshape(FC, P).T)
    bk_r = f32c(np.asarray(bk).reshape(FC, P).T)
    bv_a = f32c(bv)
    bo_a = f32c(bo)
    ones = np.ones((1, P), dtype=np.float32)
    xT = [f32c(x[b].T) for b in range(B)]

    in_maps = []
    for c in range(8):
        b, r = c // 4, c % 4
        in_maps.append({
            "xT": xT[b],
            "xqT": f32c(xT[b][:, r * QR:(r + 1) * QR]),
            "xres": f32c(x[b, r * QR:(r + 1) * QR]),
            "wqT": wqT, "wkT": wkT, "wvT": wvT, "woT": woT,
            "bq": bq_r, "bk": bk_r, "bv": bv_a, "bo": bo_a,
            "ones": ones,
        })
    return in_maps


def assemble(results):
    out = np.empty((B, S, E), dtype=np.float32)
    for c in range(8):
        b, r = c // 4, c % 4
        out[b, r * QR:(r + 1) * QR] = results[c]["out"]
    return out


def kernel(x, Wq, bq, Wk, bk, Wv, bv, Wo, bo):
    in_maps = make_in_maps(x, Wq, bq, Wk, bk, Wv, bv, Wo, bo)
    res = run_spmd(in_maps)
    return assemble(res.results)
